# revision 1
# baseline (speedup 1.0000x reference)
"""NT-Xent loss kernel for Trainium2 (8 NeuronCores, SPMD row-sharded).

Reference computation (N=4096, D=256, T=0.5):
    zi, zj = l2norm(z_i), l2norm(z_j); reps = concat([zi, zj])  # [2N, D]
    sim = reps @ reps.T
    lse_a = logsumexp over row a of sim/T with the diagonal excluded
    pos_a = sim[a, a+-N]
    loss = mean(lse_a - pos_a/T)

v3 design:
  * Row->(tile, partition) map sigma(g, p) = (g//8)*1024 + p*8 + (g%8):
    every HBM load descriptor is one contiguous 8KB run per partition.
  * Normalize writes fp8 directly (DVE tensor_scalar with fp8 output),
    so there is NO separate cast step anywhere.
  * The [rows, D] -> [D, rows] transpose runs on the DMA XBAR over fp8
    PAIRS reinterpreted as bf16 (dma_start_transpose requires a 2-byte
    dtype).  The resulting layout keeps feature pairs (2p, 2p+1) packed
    in consecutive bytes at partition p -- exactly the (partition, slot)
    pair structure the DoubleRow matmul contracts over, and the dot
    product is invariant to contraction order.  Matmul operands are
    stride-2 fp8 APs obtained via bitcast + rearrange.
  * Positive pairs and the self-similarity diagonal are computed from
    raw dots and invn scaling (no normalized bf16 copy needed): core c
    owns global tiles {c + 8k}, so tile k<4 pairs with tile k+4 locally.
  * Main loop is column-outer so it starts on the first transposed
    chunk while later chunks still stream from HBM.
"""

import sys

for _p in ("/opt/trn_rl_repo",):
    if _p not in sys.path:
        sys.path.insert(0, _p)

import numpy as np
from contextlib import ExitStack

import concourse.bass as bass
import concourse.tile as tile
from concourse import mybir
from concourse.masks import make_identity
from concourse.vector_clock import ScopedClock as _ScopedClock


def _patched_drain_and_barrier(self, tick_clock, wait_clock):
    """Tile's closing drain carries one sem-wait per DMA lane used, but this
    walrus build only accepts a single sync wait on a Drain (CTRL-NO)
    lowering ("Too many sync wait commands").  Split the waits across a
    chain of drains (sequential on SP, so semantics are unchanged)."""
    nc = self.nc
    drain_inst = nc.sync.drain()
    wait_clock.add_sem_waits(
        drain_inst.ins, _ScopedClock({None: tick_clock.global_clock})
    )
    si = drain_inst.ins.sync_info
    if si is not None:
        waits = list(si.on_wait or [])
        if len(waits) > 1:
            import bass_rust as _br

            si.on_wait = waits[:1]
            for w in waits[1:]:
                d2 = nc.sync.drain()
                d2.ins.sync_info = _br.SyncInfo(on_wait=[w], on_update=[])
    nc.all_engine_barrier()
    assert self.sems is not None
    popped = nc._tile_sem_poison_stack.pop()
    assert popped is self._sem_poison
    nc.clear_and_free_semaphores(list(self.sems.allocated().values()))
    nc.all_engine_barrier()


tile.TileContext._drain_and_barrier = _patched_drain_and_barrier

_orig_lower_ordered = tile.TileContext._lower_ordered_insts


def _split_multiwaits_and_lower(self, ordered):
    """Same walrus limitation as above, for scheduled compute/DMA
    instructions: hoist all but one sync wait onto single-wait NoOps that
    precede the instruction on its own engine."""
    nc = self.nc
    for insts in ordered.values():
        if not any(
            inst.sync_info is not None and len(inst.sync_info.on_wait or []) > 1
            for inst in insts
        ):
            continue
        out = []
        for inst in insts:
            si = inst.sync_info
            waits = list(si.on_wait) if si is not None and si.on_wait else []
            if len(waits) > 1 and getattr(inst, "engine", None) is not None:
                for w in waits[:-1]:
                    out.append(
                        mybir.InstNoOp(
                            name=nc.get_next_instruction_name(),
                            sync_info=mybir.SyncInfo(on_wait=[w], on_update=[]),
                            bass_nofuse=True,
                            engine=inst.engine,
                        )
                    )
                si.on_wait = waits[-1:]
            out.append(inst)
        insts[:] = out
    return _orig_lower_ordered(self, ordered)


tile.TileContext._lower_ordered_insts = _split_multiwaits_and_lower

N_CORES = 8
N_FULL = 4096
D_FULL = 256

f32 = mybir.dt.float32
bf16 = mybir.dt.bfloat16
fp8 = mybir.dt.float8e4
ALU = mybir.AluOpType
AF = mybir.ActivationFunctionType
AX = mybir.AxisListType


def build_bass(N=N_FULL, D=D_FULL, n_cores=N_CORES):
    n2 = 2 * N
    R = n2 // n_cores          # rows per core (1024)
    TF = n2 // 128             # full 128-row tiles (64)
    TB = R // 128              # per-core row tiles (8)
    CH = 8                     # tiles per pipeline chunk
    NCH = TF // CH             # chunks (8)
    CBW = 512                  # matmul strip width (PSUM bank)
    SBW = 2048                 # ACT superblock width (4-bank PSUM tile)
    RED1 = 16                  # stage-1 reduce group size
    TH = TB // 2

    assert R % 128 == 0 and D == 256 and TF % CH == 0

    nc = bass.Bass()
    z_i = nc.declare_dram_parameter("z_i", [N, D], f32, isOutput=False)
    z_j = nc.declare_dram_parameter("z_j", [N, D], f32, isOutput=False)
    zb = nc.declare_dram_parameter("zb", [R, D], f32, isOutput=False)
    lse_out = nc.declare_dram_parameter("lse_in", [128, TB], f32, isOutput=True)
    pos_out = nc.declare_dram_parameter("posd", [128, TH], f32, isOutput=True)

    with ExitStack() as ctx:
        tc = ctx.enter_context(tile.TileContext(nc))
        big = ctx.enter_context(tc.tile_pool(name="big", bufs=1))
        f8p = ctx.enter_context(tc.tile_pool(name="f8p", bufs=2))
        sqp = ctx.enter_context(tc.tile_pool(name="sqp", bufs=2))
        escr = ctx.enter_context(tc.tile_pool(name="escr", bufs=2))
        pmm = ctx.enter_context(tc.tile_pool(name="pmm", bufs=2, space="PSUM"))

        zf = big.tile([128, TF, D], bf16)    # all reps rows, bf16 raw
        zbn = big.tile([128, TB, D], bf16)   # this core's rows, bf16 raw
        ssq = big.tile([128, TF + TB], f32)
        lnssq = big.tile([128, TF + TB], f32)
        invn = big.tile([128, TF + TB], f32)
        # Transposed normalized fp8, stored as bf16-typed fake pairs:
        # repsTp[p, cc, t, r] (bf16) == features (2p, 2p+1) of row
        # sigma(cc*8+t, r), packed as two consecutive fp8 bytes.
        repsTp = big.tile([128, NCH, CH, 128], bf16)
        ident = big.tile([128, 128], bf16)
        make_identity(nc, ident)
        Spart = big.tile([128, TB, n2 // SBW], f32)

        def chunk_src(c):
            rows = c * (CH * 128)
            za, off = (z_i, rows) if rows < N else (z_j, rows - N)
            return za[off : off + CH * 128, :].rearrange("(p k) d -> p k d", k=CH)

        zb_r = zb[:, :].rearrange("(p k) d -> p k d", k=TB)

        def two_stage_sumsq(src, ntiles, qsl, tag):
            sq = sqp.tile([128, ntiles * D // RED1, RED1], bf16, tag=tag)
            s1 = sqp.tile([128, ntiles, D // RED1], bf16, tag=tag + "1")
            src3 = src.rearrange("p t (g r) -> p (t g) r", r=RED1)
            nc.vector.tensor_tensor(out=sq, in0=src3, in1=src3, op=ALU.mult)
            with nc.allow_low_precision("bf16 stage-1 partial sums of 16"):
                nc.vector.tensor_reduce(
                    out=s1.rearrange("p t g -> p (t g)"), in_=sq, op=ALU.add,
                    axis=AX.X,
                )
            nc.vector.reduce_sum(out=ssq[:, qsl], in_=s1, axis=AX.X)

        def chunk_dve(dst, t0, ntiles, ssq0):
            sl = slice(t0, t0 + ntiles)
            qsl = slice(ssq0, ssq0 + ntiles)
            two_stage_sumsq(dst[:, sl, :], ntiles, qsl, "sq")
            nc.scalar.activation(out=lnssq[:, qsl], in_=ssq[:, qsl], func=AF.Ln)
            nc.scalar.activation(
                out=invn[:, qsl], in_=lnssq[:, qsl], func=AF.Exp, scale=-0.5
            )
            z8 = f8p.tile([128, ntiles, D], fp8, tag="z8")
            for j in range(ntiles):
                nc.vector.tensor_scalar_mul(
                    out=z8[:, j, :], in0=dst[:, t0 + j, :],
                    scalar1=invn[:, ssq0 + j : ssq0 + j + 1],
                )
            return z8

        def chunk_pipeline(dst, t0, ntiles, ssq0, dstTp):
            """sumsq -> invn -> normalize-to-fp8 -> XBAR pair-transpose:
            out[p, t, r] = pair (2p, 2p+1) of (tile t0+t, source row r)"""
            z8 = chunk_dve(dst, t0, ntiles, ssq0)
            nc.sync.dma_start_transpose(out=dstTp, in_=z8[:, :, :].bitcast(bf16))

        def pe_transpose(z8, ntiles):
            """PE transpose of the fake-bf16 pair tiles via matmul x
            identity (bf16 -> f32 PSUM is exact); same pair layout as the
            XBAR after a cast-copy.  Used only before the main loop, so
            the PSUM pool rotation never throttles the PE."""
            tps = pmm.tile([128, SBW], f32, tag="ps")
            z8b = z8[:, :, :].bitcast(bf16)
            for t in range(ntiles):
                nc.tensor.matmul(
                    out=tps[:, t * 128 : (t + 1) * 128],
                    lhsT=z8b[:, t, :], rhs=ident,
                    start=True, stop=True,
                )
            return tps

        def pair_ap(ap_bf16):
            """[128, t, 128] fake-bf16 -> [128(K), 2(slot), t*128] fp8 AP
            for DoubleRow matmul operands."""
            return ap_bf16.bitcast(fp8).rearrange("p t (r b) -> p b (t r)", b=2)

        # ---- first loads; later loads are emitted interleaved with the
        # chunk pipelines so each XBAR transpose's conservative cross-DMA
        # ordering dep lands on an already-completed load ----
        nc.gpsimd.dma_start(out=zbn[:, :, :], in_=zb_r)
        for cc in range(2):
            nc.gpsimd.dma_start(out=zf[:, cc * CH : (cc + 1) * CH, :], in_=chunk_src(cc))

        # ---- per-core row block first: it gates every main-loop matmul.
        # PE transpose + direct PSUM repack into the slot-major contiguous
        # stationary (LDWEIGHTS dual-fp8 rejects stride-2 operands). ----
        znbT8 = big.tile([128, 2, R], fp8)
        z8zb = chunk_dve(zbn, 0, TB, TF)
        tzb = pe_transpose(z8zb, TB)
        nc.vector.tensor_copy(
            out=znbT8,
            in_=tzb.bitcast(fp8).rearrange("p (x q) -> p q x", q=4)[
                :, 2:4, 0 : TB * 128
            ],
        )

        # ---- first two zf chunks gate the main loop: PE transposes so
        # the XBAR queue starts directly on chunk 2 ----
        for cc in range(2):
            nc.gpsimd.dma_start(
                out=zf[:, (cc + 2) * CH : (cc + 3) * CH, :], in_=chunk_src(cc + 2)
            )
            z8c = chunk_dve(zf, cc * CH, CH, cc * CH)
            tpsc = pe_transpose(z8c, CH)
            nc.vector.tensor_copy(
                out=repsTp[:, cc, :, :], in_=tpsc[:, 0 : CH * 128]
            )

        # ---- self-diagonal exp(2*|zn|^2) from ssq * invn^2 (f32, [128,TB]) ----
        qz = slice(TF, TF + TB)
        d1 = big.tile([128, TB], f32)
        dacc = big.tile([128, TB], f32)
        nc.vector.tensor_mul(out=d1, in0=ssq[:, qz], in1=invn[:, qz])
        nc.vector.tensor_mul(out=dacc, in0=d1, in1=invn[:, qz])
        expd = big.tile([128, TB], f32)
        nc.scalar.activation(out=expd, in_=dacc, func=AF.Exp, scale=2.0)

        # ---- positive pairs: raw dots * invn_k * invn_{k+4}, local pairs ----
        posp = sqp.tile([128, TH, D // RED1, RED1], bf16, tag="sq")
        pos1 = sqp.tile([128, TH, D // RED1], bf16, tag="sq1")
        posr = big.tile([128, TH], f32)
        zl = zbn[:, 0:TH, :].rearrange("p t (g r) -> p t g r", r=RED1)
        zh = zbn[:, TH:TB, :].rearrange("p t (g r) -> p t g r", r=RED1)
        nc.vector.tensor_tensor(out=posp, in0=zl, in1=zh, op=ALU.mult)
        with nc.allow_low_precision("bf16 stage-1 partial sums of 16"):
            nc.vector.tensor_reduce(out=pos1, in_=posp, op=ALU.add, axis=AX.X)
        nc.vector.reduce_sum(out=posr, in_=pos1, axis=AX.X)
        ps1 = big.tile([128, TH], f32)
        ps2 = big.tile([128, TH], f32)
        posd = big.tile([128, TH], f32)
        nc.vector.tensor_mul(out=ps1, in0=posr, in1=invn[:, TF : TF + TH])
        nc.vector.tensor_mul(out=ps2, in0=ps1, in1=invn[:, TF + TH : TF + TB])
        nc.vector.tensor_scalar_mul(out=posd, in0=ps2, scalar1=2.0)
        nc.sync.dma_start(out=pos_out[:, :], in_=posd)

        # ---- remaining zf chunks (loads stay 2 chunks ahead) ----
        for cc in range(2, NCH):
            if cc + 2 < NCH:
                nc.gpsimd.dma_start(
                    out=zf[:, (cc + 2) * CH : (cc + 3) * CH, :],
                    in_=chunk_src(cc + 2),
                )
            chunk_pipeline(zf, cc * CH, CH, cc * CH, repsTp[:, cc, :, :])

        # ---- main loop: column-outer so it starts with the first chunks ----
        NSB = n2 // SBW
        MMW = SBW // CBW
        for sb in range(NSB):
            for rb in range(TB):
                ps = pmm.tile([128, SBW], f32, tag="ps")
                for j in range(MMW):
                    s = sb * MMW + j          # global 512-col strip index
                    cc, q = s // 2, s % 2     # chunk, 512-block within chunk
                    nc.tensor.matmul(
                        out=ps[:, j * CBW : (j + 1) * CBW],
                        lhsT=znbT8[:, :, rb * 128 : (rb + 1) * 128],
                        rhs=pair_ap(repsTp[:, cc, 4 * q : 4 * q + 4, :]),
                        start=True, stop=True,
                        perf_mode=mybir.MatmulPerfMode.DoubleRow,
                    )
                e = escr.tile([128, SBW], bf16, tag="e")
                nc.scalar.activation(
                    out=e, in_=ps, func=AF.Exp, scale=2.0,
                    accum_out=Spart[:, rb, sb : sb + 1],
                )

        # ---- S' = sum - diag, ship out ----
        S_t = big.tile([128, TB], f32)
        nc.vector.reduce_sum(out=S_t, in_=Spart[:, :, :], axis=AX.X)
        lse_in_t = big.tile([128, TB], f32)
        nc.vector.tensor_sub(out=lse_in_t, in0=S_t, in1=expd)
        nc.sync.dma_start(out=lse_out[:, :], in_=lse_in_t)

    return nc


_NC_CACHE = {}


def _get_nc(N=N_FULL, D=D_FULL):
    key = (N, D)
    if key not in _NC_CACHE:
        _NC_CACHE[key] = build_bass(N, D)
    return _NC_CACHE[key]


def make_in_maps(z_i, z_j, n_cores=N_CORES):
    z_i = np.ascontiguousarray(z_i, dtype=np.float32)
    z_j = np.ascontiguousarray(z_j, dtype=np.float32)
    reps = np.concatenate([z_i, z_j], axis=0)
    TB = reps.shape[0] // 128 // n_cores
    maps = []
    for c in range(n_cores):
        # core c owns global tiles {c + 8k}; tile g holds rows
        # sigma(g, p) = (g // 8) * 1024 + p * 8 + (g % 8).  zb row
        # (p*TB + k) feeds (partition p, local tile k).
        idx = np.empty(128 * TB, dtype=np.int64)
        for p in range(128):
            for k in range(TB):
                idx[p * TB + k] = k * 1024 + p * 8 + c
        maps.append({"z_i": z_i, "z_j": z_j, "zb": np.ascontiguousarray(reps[idx])})
    return maps


def assemble(results, N=N_FULL, n_cores=N_CORES):
    """Host-side gather + final ln/mean ("all-reduce the mean loss")."""
    n2 = 2 * N
    TB = n2 // 128 // n_cores
    lse = np.empty(n2, dtype=np.float64)
    pos = np.empty(n2, dtype=np.float64)
    p_ar = np.arange(128)
    for c, r in enumerate(results):
        lse_in = np.asarray(r["lse_in"], dtype=np.float64)   # [128, TB]
        posd = np.asarray(r["posd"], dtype=np.float64)       # [128, TB//2]
        for k in range(TB):
            rows = k * 1024 + p_ar * 8 + c   # sigma(c + 8k, p)
            lse[rows] = lse_in[:, k]
            pos[rows] = posd[:, k % (TB // 2)]
    loss = np.mean(np.log(lse) - pos)
    return np.float32(loss)


def _run(z_i, z_j, trace=False, tmpdir=None, **spmd_kwargs):
    from concourse.bass_utils import run_bass_kernel_spmd

    N, D = z_i.shape
    nc = _get_nc(N, D)
    in_maps = make_in_maps(z_i, z_j)
    out = run_bass_kernel_spmd(
        nc, in_maps, list(range(N_CORES)), trace=trace, tmpdir=tmpdir, **spmd_kwargs
    )
    return assemble(out.results, N), out


def kernel(z_i, z_j):
    loss, _ = _run(np.asarray(z_i), np.asarray(z_j))
    return loss


if __name__ == "__main__":
    rng = np.random.default_rng(0)
    z_i = rng.standard_normal((N_FULL, D_FULL), dtype=np.float32)
    z_j = rng.standard_normal((N_FULL, D_FULL), dtype=np.float32)
    print(kernel(z_i, z_j))

